# revision 1
# baseline (speedup 1.0000x reference)
"""RGCN (mean-aggregation) message-passing kernel for 8 Trainium2 NeuronCores.

Problem shapes (hardcoded):
  B=16, L=512, H=256, R=8, E=524288, N = B*2*L = 16384 nodes.

Strategy (dst-sharded, no collectives):
  - Host: node features x = concat(input_s, input_a) -> [N, H]. Edges are
    bucketed by destination-owner core (2048 dst nodes per core), then sorted
    by local segment id  lseg = rel*2048 + local_dst  (relation-major).
    Each core's 16384 segments split into 128 blocks of 128 segments. Every
    block is padded to T_b tiles of 128 edges (uniform static program).
  - Device, per 128-edge tile: indirect-DMA gather of the 128 source rows
    from the replicated HBM node table; build a one-hot selection matrix
    S[p, q] = (slot_p == q) with one DVE is_equal against an iota constant;
    matmul  psum[128 seg, 256 h] += S.T @ G  accumulates the block's
    segment-sum in PSUM. Block result is scaled by 1/count (mean) into SBUF.
  - Mean tiles are PE-transposed, then per-relation GEMMs against W_r plus
    the root GEMM against x^T accumulate out^T [256, 2048] per core.
  - Host reassembles [N, H], splits into (sent, act).
"""

import sys

if "/opt/trn_rl_repo" not in sys.path:
    sys.path.insert(0, "/opt/trn_rl_repo")

import numpy as np

B, L, H, R = 16, 512, 256, 8
N = B * 2 * L          # 16384 nodes
E = 524288
NCORES = 8
NPC = N // NCORES      # 2048 nodes per core
SEGS = NPC * R         # 16384 segments per core
NBLK = SEGS // 128     # 128 blocks per core
P = 128

_COMPILED = {}         # T_b -> (nc, names)


def _build_program(T_b):
    """Build + compile the 8-core SPMD Bass program for tile depth T_b."""
    from concourse import bass, bacc, tile, mybir
    from concourse.masks import make_identity

    f32 = mybir.dt.float32
    i32 = mybir.dt.int32
    T = NBLK * T_b

    nc = bacc.Bacc("TRN2", target_bir_lowering=False, debug=False,
                   num_devices=NCORES)

    bf16 = mybir.dt.bfloat16
    xtab = nc.dram_tensor("xtab", [N, 2 * H], bf16, kind="ExternalInput")
    srcs = nc.dram_tensor("srcs", [P, T], i32, kind="ExternalInput")
    iotad = nc.dram_tensor("iotad", [P, P], f32, kind="ExternalInput")
    identd = nc.dram_tensor("identd", [P, P], f32, kind="ExternalInput")
    slots = nc.dram_tensor("slots", [P, T], f32, kind="ExternalInput")
    recip = nc.dram_tensor("recip", [P, NBLK], f32, kind="ExternalInput")
    wt = nc.dram_tensor("wt", [P, R * 2 * 2 * P], f32, kind="ExternalInput")
    roott = nc.dram_tensor("roott", [P, 2 * 2 * P], f32, kind="ExternalInput")
    biast = nc.dram_tensor("biast", [P, 2], f32, kind="ExternalInput")
    xt = nc.dram_tensor("xt", [P, 2 * NPC], f32, kind="ExternalInput")
    out = nc.dram_tensor("out", [H, NPC], f32, kind="ExternalOutput")

    with tile.TileContext(nc) as tc:
        with (
            tc.tile_pool(name="const", bufs=1) as cpool,
            tc.tile_pool(name="g", bufs=12) as gpool,
            tc.tile_pool(name="s", bufs=12) as spool,
            tc.tile_pool(name="psb", bufs=3, space="PSUM") as psb_pool,
            tc.tile_pool(name="acc", bufs=34) as accpool,
            tc.tile_pool(name="pt", bufs=3, space="PSUM") as pt_pool,
            tc.tile_pool(name="mt", bufs=2) as mtpool,
            tc.tile_pool(name="po", bufs=2, space="PSUM") as po_pool,
        ):
            srcs_sb = cpool.tile([P, T], i32)
            nc.sync.dma_start(srcs_sb[:], srcs.ap())
            slots_sb = cpool.tile([P, T], f32)
            nc.sync.dma_start(slots_sb[:], slots.ap())
            recip_sb = cpool.tile([P, NBLK], f32)
            nc.sync.dma_start(recip_sb[:], recip.ap())
            w_sb = cpool.tile([P, R * 2 * 2 * P], f32)
            nc.sync.dma_start(w_sb[:], wt.ap())
            root_sb = cpool.tile([P, 2 * 2 * P], f32)
            nc.sync.dma_start(root_sb[:], roott.ap())
            bias_sb = cpool.tile([P, 2], f32)
            nc.sync.dma_start(bias_sb[:], biast.ap())
            xt_sb = cpool.tile([P, 2 * NPC], f32)
            nc.sync.dma_start(xt_sb[:], xt.ap())

            iota_f = cpool.tile([P, P], f32)
            nc.sync.dma_start(iota_f[:], iotad.ap())
            ident = cpool.tile([P, P], f32)
            nc.sync.dma_start(ident[:], identd.ap())

            outacc = cpool.tile([P, 2, NPC], f32)

            for r in range(R):
                acc_tiles = []
                for nb in range(16):
                    b = r * 16 + nb
                    ps = psb_pool.tile([P, H], f32)
                    for j in range(T_b):
                        t = b * T_b + j
                        g = gpool.tile([P, 2 * H], bf16)
                        nc.gpsimd.indirect_dma_start(
                            out=g[:], out_offset=None, in_=xtab.ap(),
                            in_offset=bass.IndirectOffsetOnAxis(
                                ap=srcs_sb[:, t:t + 1], axis=0))
                        s = spool.tile([P, P], bf16)
                        nc.vector.tensor_scalar(
                            out=s[:], in0=iota_f[:],
                            scalar1=slots_sb[:, t:t + 1], scalar2=None,
                            op0=mybir.AluOpType.is_equal)
                        # hi + lo bf16 halves accumulate exactly in f32 PSUM
                        nc.tensor.matmul(out=ps[:], lhsT=s[:],
                                         rhs=g[:, 0:H],
                                         start=(j == 0), stop=False)
                        nc.tensor.matmul(out=ps[:], lhsT=s[:],
                                         rhs=g[:, H:2 * H],
                                         start=False, stop=(j == T_b - 1))
                    a = accpool.tile([P, H], f32)
                    nc.vector.tensor_scalar(
                        out=a[:], in0=ps[:], scalar1=recip_sb[:, b:b + 1],
                        scalar2=None, op0=mybir.AluOpType.mult)
                    acc_tiles.append(a)

                # transpose mean_r [2048 n, 256 k] -> mt [128 kpart, 2 kc, 2048 n]
                mt = mtpool.tile([P, 2, NPC], f32)
                for kc in range(2):
                    for nb in range(16):
                        pt = pt_pool.tile([P, P], f32)
                        nc.tensor.transpose(
                            out=pt[:],
                            in_=acc_tiles[nb][:, kc * P:(kc + 1) * P],
                            identity=ident[:])
                        nc.vector.tensor_copy(
                            out=mt[:, kc, nb * P:(nb + 1) * P], in_=pt[:])

                # GEMM: out^T[mc, :] += W_r[:, mc].T-chunks @ mean_r^T
                for mc in range(2):
                    for n4 in range(4):
                        po = po_pool.tile([P, 512], f32)
                        for kc in range(2):
                            wofs = ((r * 2 + kc) * 2 + mc) * P
                            nc.tensor.matmul(
                                out=po[:],
                                lhsT=w_sb[:, wofs:wofs + P],
                                rhs=mt[:, kc, n4 * 512:(n4 + 1) * 512],
                                start=(kc == 0), stop=(kc == 1))
                        osl = outacc[:, mc, n4 * 512:(n4 + 1) * 512]
                        if r == 0:
                            nc.vector.tensor_copy(out=osl, in_=po[:])
                        else:
                            nc.vector.tensor_add(out=osl, in0=osl, in1=po[:])

            # root GEMM: out^T += root^T-chunks @ x^T
            for mc in range(2):
                for n4 in range(4):
                    po = po_pool.tile([P, 512], f32)
                    for kc in range(2):
                        rofs = (kc * 2 + mc) * P
                        nc.tensor.matmul(
                            out=po[:],
                            lhsT=root_sb[:, rofs:rofs + P],
                            rhs=xt_sb[:, kc * NPC + n4 * 512:
                                      kc * NPC + (n4 + 1) * 512],
                            start=(kc == 0), stop=(kc == 1))
                    osl = outacc[:, mc, n4 * 512:(n4 + 1) * 512]
                    nc.vector.tensor_add(out=osl, in0=osl, in1=po[:])

            for mc in range(2):
                nc.vector.tensor_scalar(
                    out=outacc[:, mc, :], in0=outacc[:, mc, :],
                    scalar1=bias_sb[:, mc:mc + 1], scalar2=None,
                    op0=mybir.AluOpType.add)
                nc.sync.dma_start(out.ap()[mc * P:(mc + 1) * P, :],
                                  outacc[:, mc, :])

    nc.compile()
    return nc


def _prep_inputs(input_s, input_a, edge_index, edge_type, weight, root, bias):
    """Host-side sharding/layout prep. Returns (T_b, in_maps)."""
    import ml_dtypes
    x = np.ascontiguousarray(
        np.concatenate([input_s, input_a], axis=1).reshape(N, H)
    ).astype(np.float32)
    x_hi = x.astype(ml_dtypes.bfloat16)
    x_lo = (x - x_hi.astype(np.float32)).astype(ml_dtypes.bfloat16)
    xtab_hl = np.ascontiguousarray(np.concatenate([x_hi, x_lo], axis=1))

    src = np.asarray(edge_index[0]).astype(np.int64)
    dst = np.asarray(edge_index[1]).astype(np.int64)
    et = np.asarray(edge_type).astype(np.int64)

    cnt = np.bincount(dst * R + et, minlength=N * R).reshape(N, R)
    recip_full = (1.0 / np.maximum(cnt, 1)).astype(np.float32)  # [N, R]

    owner = dst // NPC
    lseg = et * NPC + (dst - owner * NPC)          # relation-major local seg
    key = owner * SEGS + lseg
    order = np.argsort(key, kind="stable")
    sk = key[order]
    ssrc = src[order].astype(np.int32)

    bg = sk >> 7                                   # global block id [0, 1024)
    counts_bg = np.bincount(bg, minlength=NCORES * NBLK)
    T_b = int(np.ceil(counts_bg.max() / P))
    cap = T_b * P
    starts = np.concatenate([[0], np.cumsum(counts_bg)])
    pos = np.arange(E) - starts[bg]
    dest = bg * cap + pos

    srcs_pad = np.zeros(NCORES * NBLK * cap, np.int32)
    slots_pad = np.full(NCORES * NBLK * cap, -1.0, np.float32)
    # Dummy (padding) entries sit at each block's tail: encode as -1 so the
    # dma_gather ucode skips them (no descriptor cost). The first 4 blocks of
    # each core keep real row-0 gathers so every G-pool slot's first use
    # fully writes the buffer (a skipped row leaves stale SBUF; stale-NaN * 0
    # in the matmul would poison PSUM).
    srcs_pad[dest] = ssrc
    slots_pad[dest] = (sk & 127).astype(np.float32)
    srcs_c = srcs_pad.reshape(NCORES, NBLK * T_b, P).transpose(0, 2, 1)
    slots_c = slots_pad.reshape(NCORES, NBLK * T_b, P).transpose(0, 2, 1)
    iota_host = np.broadcast_to(np.arange(P, dtype=np.float32), (P, P)).copy()
    ident_host = np.eye(P, dtype=np.float32)

    w_host = np.ascontiguousarray(
        np.asarray(weight, np.float32).reshape(R, 2, P, 2, P)
        .transpose(2, 0, 1, 3, 4).reshape(P, R * 2 * 2 * P))
    root_host = np.ascontiguousarray(
        np.asarray(root, np.float32).reshape(2, P, 2, P)
        .transpose(1, 0, 2, 3).reshape(P, 2 * 2 * P))
    bias_host = np.ascontiguousarray(
        np.asarray(bias, np.float32).reshape(2, P).T)

    in_maps = []
    for c in range(NCORES):
        xc = x[c * NPC:(c + 1) * NPC]              # [2048, 256]
        xt_host = np.ascontiguousarray(
            xc.T.reshape(2, P, NPC).transpose(1, 0, 2).reshape(P, 2 * NPC))
        rc = recip_full[c * NPC:(c + 1) * NPC, :].T.reshape(SEGS)
        recip_host = np.ascontiguousarray(rc.reshape(NBLK, P).T)
        in_maps.append({
            "xtab": xtab_hl,
            "srcs": np.ascontiguousarray(srcs_c[c]),
            "slots": np.ascontiguousarray(slots_c[c]),
            "recip": recip_host,
            "wt": w_host,
            "roott": root_host,
            "biast": bias_host,
            "xt": xt_host,
            "iotad": iota_host,
            "identd": ident_host,
        })
    return T_b, in_maps


def _run(in_maps, T_b, trace=False, trace_cores=None):
    from concourse import bass_utils
    if T_b not in _COMPILED:
        _COMPILED[T_b] = _build_program(T_b)
    nc = _COMPILED[T_b]
    kwargs = {}
    if trace:
        _install_ntff_shim()
        bass_utils.upload_artifacts = lambda tmpdir: tmpdir
        kwargs = dict(trace=True,
                      trace_cores=trace_cores if trace_cores else [0])
    return bass_utils.run_bass_kernel_spmd(
        nc, in_maps, core_ids=list(range(NCORES)), **kwargs)


def _assemble(results):
    full = np.empty((N, H), np.float32)
    for c in range(NCORES):
        full[c * NPC:(c + 1) * NPC, :] = results[c]["out"].T
    dtrp = full.reshape(B, 2 * L, H)
    sent = np.ascontiguousarray(dtrp[:, :L, :])
    act = np.ascontiguousarray(dtrp[:, L:, :])
    return sent, act


def kernel(input_s, input_a, edge_index, edge_type, weight, root, bias,
           _trace=False, _trace_cores=None, _return_stats=False):
    T_b, in_maps = _prep_inputs(input_s, input_a, edge_index, edge_type,
                                weight, root, bias)
    res = _run(in_maps, T_b, trace=_trace, trace_cores=_trace_cores)
    out = _assemble(res.results)
    if _return_stats:
        return out, res
    return out


def _install_ntff_shim():
    """Install antenv.axon_hooks NTFF profiling hook via ctypes (the agent
    image lacks the module; same mechanism trn_boot would use)."""
    import types, ctypes, contextlib
    if "antenv.axon_hooks" in sys.modules:
        return
    so_path = "/opt/axon/libaxon_pjrt.so"
    lib = ctypes.CDLL(so_path)
    if not hasattr(lib, "axon_start_nrt_profile"):
        return
    lib.axon_start_nrt_profile.argtypes = [ctypes.POINTER(ctypes.c_int64),
                                           ctypes.c_size_t]
    lib.axon_start_nrt_profile.restype = ctypes.c_int64
    lib.axon_stop_nrt_profile.argtypes = [ctypes.c_char_p]
    lib.axon_stop_nrt_profile.restype = ctypes.c_int64

    @contextlib.contextmanager
    def _hook(output_dir, device_ids):
        import jax
        jax.devices()
        if device_ids:
            ids = (ctypes.c_int64 * len(device_ids))(*device_ids)
            rc = lib.axon_start_nrt_profile(ids, len(device_ids))
        else:
            rc = lib.axon_start_nrt_profile(None, 0)
        if rc != 0:
            raise RuntimeError(f"axon_start_nrt_profile rc={rc}")
        try:
            yield
        finally:
            n = lib.axon_stop_nrt_profile(str(output_dir).encode())
            if n < 0:
                raise RuntimeError(f"axon_stop_nrt_profile rc={n}")

    import antenv
    mod = types.ModuleType("antenv.axon_hooks")
    mod.get_axon_ntff_profile_hook = lambda: _hook
    mod.set_axon_ntff_profile_hook = lambda h: None
    sys.modules["antenv.axon_hooks"] = mod
    antenv.axon_hooks = mod



# revision 8
# speedup vs baseline: 1.4507x; 1.4507x over previous
"""RGCN (mean-aggregation) message-passing kernel for 8 Trainium2 NeuronCores.

Problem shapes (hardcoded):
  B=16, L=512, H=256, R=8, E=524288, N = B*2*L = 16384 nodes.

Strategy (dst-sharded, no collectives):
  - Host: node features x = concat(input_s, input_a) -> [N, H], stored bf16.
    Edges are bucketed by destination-owner core (2048 dst nodes per core),
    grouped by local segment id  lseg = rel*2048 + local_dst  (relation-major).
    Each core's 16384 segments split into 128 blocks of 128 segments. Every
    block is padded to T_b tiles of 128 edges (uniform static program);
    within a block edges are sorted by source row for HBM locality.
  - Device: ONE dma_gather per 16 tiles (2048 idxs, int16 wrapped table,
    single_packet=False -- the single-packet path caps at 64 descriptors
    per engine = 1024 idxs) pulls rows from the bf16 HBM node table,
    amortizing the ~1us SWDGE fixed cost that dominated the per-tile
    indirect-DMA version.
  - Per 128-edge tile: build one-hot S[p, q] = (slot_p == q) on DVE (bf16);
    matmul  psum[128 seg, 256 h] += S.T @ G  accumulates the block's
    segment-sum in PSUM (bf16 operands, f32 accumulate). Block result is
    scaled by 1/count (mean) into SBUF f32 on the Activation engine.
  - Mean tiles are PE-transposed (f32), copied to mt on ACT, then per-relation
    GEMMs in fp32r (1 cycle/row at free-dim 512) against W_r plus the root
    GEMM against x^T accumulate out^T [256, 2048] per core.
  - Host reassembles [N, H], splits into (sent, act).
"""

import sys

if "/opt/trn_rl_repo" not in sys.path:
    sys.path.insert(0, "/opt/trn_rl_repo")

import numpy as np

B, L, H, R = 16, 512, 256, 8
N = B * 2 * L          # 16384 nodes
E = 524288
NCORES = 8
NPC = N // NCORES      # 2048 nodes per core
SEGS = NPC * R         # 16384 segments per core
NBLK = SEGS // 128     # 128 blocks per core
P = 128
NI_G = 2048            # idxs per dma_gather (needs single_packet=False)

_COMPILED = {}         # T_b -> nc


def _build_program(T_b):
    """Build + compile the 8-core SPMD Bass program for tile depth T_b."""
    from concourse import bacc, bass, tile, mybir

    f32 = mybir.dt.float32
    f32r = mybir.dt.float32r
    bf16 = mybir.dt.bfloat16
    i16 = mybir.dt.int16
    T = NBLK * T_b                  # total 128-edge tiles per core
    TPG = NI_G // P                 # tiles per gather
    IC = NI_G // 16                 # idx columns per gather
    NG = T // TPG                   # gathers per core

    nc = bacc.Bacc("TRN2", target_bir_lowering=False, debug=False,
                   num_devices=NCORES)

    xtab = nc.dram_tensor("xtab", [N, H], bf16, kind="ExternalInput")
    idxs = nc.dram_tensor("idxs", [P, NG * IC], i16, kind="ExternalInput")
    slots = nc.dram_tensor("slots", [P, T], f32, kind="ExternalInput")
    recip = nc.dram_tensor("recip", [P, NBLK], f32, kind="ExternalInput")
    iotad = nc.dram_tensor("iotad", [P, P], bf16, kind="ExternalInput")
    identd = nc.dram_tensor("identd", [P, P], f32, kind="ExternalInput")
    wt = nc.dram_tensor("wt", [P, R * 2 * 2 * P], f32r, kind="ExternalInput")
    roott = nc.dram_tensor("roott", [P, 2 * 2 * P], f32r, kind="ExternalInput")
    biast = nc.dram_tensor("biast", [P, 2], f32, kind="ExternalInput")
    xt = nc.dram_tensor("xt", [P, 2 * NPC], f32r, kind="ExternalInput")
    out = nc.dram_tensor("out", [H, NPC], f32, kind="ExternalOutput")

    with tile.TileContext(nc) as tc:
        with (
            tc.tile_pool(name="const", bufs=1) as cpool,
            tc.tile_pool(name="g", bufs=4) as gpool,
            tc.tile_pool(name="s", bufs=12) as spool,
            tc.tile_pool(name="psb", bufs=4, space="PSUM") as psb_pool,
            tc.tile_pool(name="acc", bufs=34) as accpool,
            tc.tile_pool(name="pt", bufs=2, space="PSUM") as pt_pool,
            tc.tile_pool(name="mt", bufs=2) as mtpool,
            tc.tile_pool(name="po", bufs=2, space="PSUM") as po_pool,
        ):
            idxs_sb = cpool.tile([P, NG * IC], i16)
            nc.sync.dma_start(idxs_sb[:], idxs.ap())
            slots_sb = cpool.tile([P, T], f32)
            nc.sync.dma_start(slots_sb[:], slots.ap())
            recip_sb = cpool.tile([P, NBLK], f32)
            nc.sync.dma_start(recip_sb[:], recip.ap())
            w_sb = cpool.tile([P, R * 2 * 2 * P], f32r)
            nc.sync.dma_start(w_sb[:], wt.ap())
            root_sb = cpool.tile([P, 2 * 2 * P], f32r)
            nc.sync.dma_start(root_sb[:], roott.ap())
            bias_sb = cpool.tile([P, 2], f32)
            nc.sync.dma_start(bias_sb[:], biast.ap())
            xt_sb = cpool.tile([P, 2 * NPC], f32r)
            nc.sync.dma_start(xt_sb[:], xt.ap())

            iota_bf = cpool.tile([P, P], bf16)
            nc.sync.dma_start(iota_bf[:], iotad.ap())
            ident = cpool.tile([P, P], f32)
            nc.sync.dma_start(ident[:], identd.ap())

            outacc = cpool.tile([P, 2, NPC], f32)

            for r in range(R):
                acc_tiles = []
                for nb in range(16):
                    b = r * 16 + nb
                    ps = psb_pool.tile([P, H], f32)
                    for j in range(T_b):
                        t = b * T_b + j
                        if t % TPG == 0:
                            gi = t // TPG
                            g_cur = gpool.tile([P, TPG, H], bf16)
                            nc.gpsimd.dma_gather(
                                g_cur[:], xtab.ap(),
                                idxs_sb[:, gi * IC:(gi + 1) * IC],
                                NI_G, NI_G, H, single_packet=False)
                        s = spool.tile([P, P], bf16)
                        nc.vector.tensor_scalar(
                            out=s[:], in0=iota_bf[:],
                            scalar1=slots_sb[:, t:t + 1], scalar2=None,
                            op0=mybir.AluOpType.is_equal)
                        nc.tensor.matmul(
                            out=ps[:], lhsT=s[:],
                            rhs=g_cur[:, t % TPG, :],
                            start=(j == 0), stop=(j == T_b - 1))
                    a = accpool.tile([P, H], f32)
                    nc.scalar.mul(a[:], ps[:], recip_sb[:, b:b + 1])
                    acc_tiles.append(a)

                # transpose mean_r [2048 n, 256 k] -> mt [128 kpart, 2 kc, 2048 n]
                mt = mtpool.tile([P, 2, NPC], f32r)
                for kc in range(2):
                    for nb in range(16):
                        pt = pt_pool.tile([P, P], f32)
                        nc.tensor.transpose(
                            out=pt[:],
                            in_=acc_tiles[nb][:, kc * P:(kc + 1) * P],
                            identity=ident[:])
                        nc.scalar.copy(mt[:, kc, nb * P:(nb + 1) * P], pt[:])

                # GEMM (fp32r): out^T[mc, :] += W_r[:, mc].T-chunks @ mean_r^T
                for mc in range(2):
                    for n4 in range(4):
                        po = po_pool.tile([P, 512], f32)
                        for kc in range(2):
                            wofs = ((r * 2 + kc) * 2 + mc) * P
                            nc.tensor.matmul(
                                out=po[:],
                                lhsT=w_sb[:, wofs:wofs + P],
                                rhs=mt[:, kc, n4 * 512:(n4 + 1) * 512],
                                start=(kc == 0), stop=(kc == 1))
                        osl = outacc[:, mc, n4 * 512:(n4 + 1) * 512]
                        if r == 0:
                            nc.vector.tensor_copy(out=osl, in_=po[:])
                        else:
                            nc.vector.tensor_add(out=osl, in0=osl, in1=po[:])

            # root GEMM (fp32r): out^T += root^T-chunks @ x^T
            for mc in range(2):
                for n4 in range(4):
                    po = po_pool.tile([P, 512], f32)
                    for kc in range(2):
                        rofs = (kc * 2 + mc) * P
                        nc.tensor.matmul(
                            out=po[:],
                            lhsT=root_sb[:, rofs:rofs + P],
                            rhs=xt_sb[:, kc * NPC + n4 * 512:
                                      kc * NPC + (n4 + 1) * 512],
                            start=(kc == 0), stop=(kc == 1))
                    osl = outacc[:, mc, n4 * 512:(n4 + 1) * 512]
                    nc.vector.tensor_add(out=osl, in0=osl, in1=po[:])

            for mc in range(2):
                nc.vector.tensor_scalar(
                    out=outacc[:, mc, :], in0=outacc[:, mc, :],
                    scalar1=bias_sb[:, mc:mc + 1], scalar2=None,
                    op0=mybir.AluOpType.add)
                nc.sync.dma_start(out.ap()[mc * P:(mc + 1) * P, :],
                                  outacc[:, mc, :])

    nc.compile()
    return nc


def _prep_inputs(input_s, input_a, edge_index, edge_type, weight, root, bias):
    """Host-side sharding/layout prep. Returns (T_b, in_maps)."""
    import ml_dtypes
    x = np.ascontiguousarray(
        np.concatenate([input_s, input_a], axis=1).reshape(N, H)
    ).astype(np.float32)
    xtab_bf = np.ascontiguousarray(x.astype(ml_dtypes.bfloat16))

    src = np.asarray(edge_index[0]).astype(np.int64)
    dst = np.asarray(edge_index[1]).astype(np.int64)
    et = np.asarray(edge_type).astype(np.int64)

    cnt = np.bincount(dst * R + et, minlength=N * R).reshape(N, R)
    recip_full = (1.0 / np.maximum(cnt, 1)).astype(np.float32)  # [N, R]

    owner = dst // NPC
    lseg = et * NPC + (dst - owner * NPC)          # relation-major local seg
    key = owner * SEGS + lseg
    # sort by (block, src) so gather addresses ascend within each block
    bg_of_key = key >> 7
    order = np.lexsort((src, bg_of_key))
    sk = key[order]
    ssrc = src[order].astype(np.int16)

    bg = sk >> 7                                   # global block id [0, 1024)
    counts_bg = np.bincount(bg, minlength=NCORES * NBLK)
    T_b = int(np.ceil(counts_bg.max() / P))
    cap = T_b * P
    starts = np.concatenate([[0], np.cumsum(counts_bg)])
    pos = np.arange(E) - starts[bg]
    dest = bg * cap + pos

    # Padding slots keep src=0 (all descriptors valid; repeated row-0 reads
    # are HBM row-buffer hits) and slot=-1 (matches no one-hot column).
    srcs_pad = np.zeros(NCORES * NBLK * cap, np.int16)
    slots_pad = np.full(NCORES * NBLK * cap, -1.0, np.float32)
    srcs_pad[dest] = ssrc
    slots_pad[dest] = (sk & 127).astype(np.float32)
    srcs_c = srcs_pad.reshape(NCORES, NBLK * T_b * P)
    slots_c = slots_pad.reshape(NCORES, NBLK * T_b, P).transpose(0, 2, 1)
    iota_host = np.broadcast_to(
        np.arange(P, dtype=np.float32), (P, P)).astype(ml_dtypes.bfloat16)
    ident_host = np.eye(P, dtype=np.float32)

    w_host = np.ascontiguousarray(
        np.asarray(weight, np.float32).reshape(R, 2, P, 2, P)
        .transpose(2, 0, 1, 3, 4).reshape(P, R * 2 * 2 * P))
    root_host = np.ascontiguousarray(
        np.asarray(root, np.float32).reshape(2, P, 2, P)
        .transpose(1, 0, 2, 3).reshape(P, 2 * 2 * P))
    bias_host = np.ascontiguousarray(
        np.asarray(bias, np.float32).reshape(2, P).T)

    in_maps = []
    for c in range(NCORES):
        xc = x[c * NPC:(c + 1) * NPC]              # [2048, 256]
        xt_host = np.ascontiguousarray(
            xc.T.reshape(2, P, NPC).transpose(1, 0, 2).reshape(P, 2 * NPC))
        rc = recip_full[c * NPC:(c + 1) * NPC, :].T.reshape(SEGS)
        recip_host = np.ascontiguousarray(rc.reshape(NBLK, P).T)
        # idx i of the flat stream sits at wrapped[i%16, i//16]; replicated
        # 8x across the 128 partitions for the gather ucode subcores.
        idx_wrapped = np.ascontiguousarray(
            np.tile(srcs_c[c].reshape(-1, 16).T, (8, 1)))
        in_maps.append({
            "xtab": xtab_bf,
            "idxs": idx_wrapped,
            "slots": np.ascontiguousarray(slots_c[c]),
            "recip": recip_host,
            "wt": w_host,
            "roott": root_host,
            "biast": bias_host,
            "xt": xt_host,
            "iotad": np.ascontiguousarray(iota_host),
            "identd": ident_host,
        })
    return T_b, in_maps


def _run(in_maps, T_b, trace=False, trace_cores=None):
    from concourse import bass_utils
    if T_b not in _COMPILED:
        _COMPILED[T_b] = _build_program(T_b)
    nc = _COMPILED[T_b]
    kwargs = {}
    if trace:
        _install_ntff_shim()
        bass_utils.upload_artifacts = lambda tmpdir: tmpdir
        kwargs = dict(trace=True,
                      trace_cores=trace_cores if trace_cores else [0])
    return bass_utils.run_bass_kernel_spmd(
        nc, in_maps, core_ids=list(range(NCORES)), **kwargs)


def _assemble(results):
    full = np.empty((N, H), np.float32)
    for c in range(NCORES):
        full[c * NPC:(c + 1) * NPC, :] = results[c]["out"].T
    dtrp = full.reshape(B, 2 * L, H)
    sent = np.ascontiguousarray(dtrp[:, :L, :])
    act = np.ascontiguousarray(dtrp[:, L:, :])
    return sent, act


def kernel(input_s, input_a, edge_index, edge_type, weight, root, bias,
           _trace=False, _trace_cores=None, _return_stats=False):
    T_b, in_maps = _prep_inputs(input_s, input_a, edge_index, edge_type,
                                weight, root, bias)
    res = _run(in_maps, T_b, trace=_trace, trace_cores=_trace_cores)
    out = _assemble(res.results)
    if _return_stats:
        return out, res
    return out


def _install_ntff_shim():
    """Install antenv.axon_hooks NTFF profiling hook via ctypes (the agent
    image lacks the module; same mechanism trn_boot would use)."""
    import types, ctypes, contextlib
    if "antenv.axon_hooks" in sys.modules:
        return
    so_path = "/opt/axon/libaxon_pjrt.so"
    lib = ctypes.CDLL(so_path)
    if not hasattr(lib, "axon_start_nrt_profile"):
        return
    lib.axon_start_nrt_profile.argtypes = [ctypes.POINTER(ctypes.c_int64),
                                           ctypes.c_size_t]
    lib.axon_start_nrt_profile.restype = ctypes.c_int64
    lib.axon_stop_nrt_profile.argtypes = [ctypes.c_char_p]
    lib.axon_stop_nrt_profile.restype = ctypes.c_int64

    @contextlib.contextmanager
    def _hook(output_dir, device_ids):
        import jax
        jax.devices()
        if device_ids:
            ids = (ctypes.c_int64 * len(device_ids))(*device_ids)
            rc = lib.axon_start_nrt_profile(ids, len(device_ids))
        else:
            rc = lib.axon_start_nrt_profile(None, 0)
        if rc != 0:
            raise RuntimeError(f"axon_start_nrt_profile rc={rc}")
        try:
            yield
        finally:
            n = lib.axon_stop_nrt_profile(str(output_dir).encode())
            if n < 0:
                raise RuntimeError(f"axon_stop_nrt_profile rc={n}")

    import antenv
    mod = types.ModuleType("antenv.axon_hooks")
    mod.get_axon_ntff_profile_hook = lambda: _hook
    mod.set_axon_ntff_profile_hook = lambda h: None
    sys.modules["antenv.axon_hooks"] = mod
    antenv.axon_hooks = mod
